# revision 51
# baseline (speedup 1.0000x reference)
"""Multi-head self-attention (B=2, S=2048, E=1024, H=16) on 8 Trainium2 cores.

Sharding: 2D (batch x head-group). Core c handles batch b = c // 4 and head
group g = c % 4 (4 heads, 256 embed columns). Each core computes its QKV
projection slices, fused attention for its 4 heads, and a partial output
projection (attn_g @ Wo[g_slice]); the host sums the 4 partials per batch
(the head-concat contraction) and stacks the 2 batches.

Device layout choices (all matmul contractions land on the partition axis,
so no on-device transposes are needed anywhere):
  - host supplies x^T per batch for q/k/v, bf16, pre-tiled into the exact
    [128, KC, span] blocks the kernel loads (every DMA fully contiguous)
  - Q/K projections produce Q^T/K^T  [d', S] (head-dim on partitions)
  - V projection produces V [S, d'] (seq on partitions), stored interleaved
    with a ones column per head ([V_h | 1] * 4) so that P @ [V_h | 1] yields
    both the attention numerator and the softmax denominator in one pass
  - logits^T tiles [j, i] feed exp (ScalarE, no max-subtraction: |logits|<~8)
    giving P^T tiles which are exactly the rhs layout P@V needs
  - 1/8 scaling and biases are folded in on the host / into copy-backs;
    bv is folded via P @ [V + 1 bv^T] = P@V + bv (softmax rows sum to 1)
"""

import numpy as np
import ml_dtypes

BF16 = ml_dtypes.bfloat16

P = 128
S = 2048
E = 1024
GE = 256          # embed columns per core (4 heads x 64)
KC = 8            # contraction chunks of 128 over E
JC = 16           # key chunks of 128 over S
IT = 4            # query tiles of 512 over S
NCORES = 8

_NC = None        # cached compiled program


def _build_program():
    import concourse.tile as tile
    from concourse import bacc, mybir

    F32 = mybir.dt.float32
    BF = mybir.dt.bfloat16
    Exp = mybir.ActivationFunctionType.Exp
    mult = mybir.AluOpType.mult
    add = mybir.AluOpType.add

    nc = bacc.Bacc(
        "TRN2",
        target_bir_lowering=False,
        debug=False,
        enable_asserts=False,
        num_devices=NCORES,
    )

    # x inputs come pre-tiled from the host so every SBUF load is one
    # fully contiguous DMA: [g, p, kc, s] = x^T[kc*128+p, g*W+s]
    d_xq = nc.dram_tensor("xqT", [4, P, KC, 512], BF, kind="ExternalInput")
    d_xk = nc.dram_tensor("xkT", [4, P, KC, 512], BF, kind="ExternalInput")
    d_xv = nc.dram_tensor("xvT", [8, P, KC, GE], BF, kind="ExternalInput")
    d_wq = nc.dram_tensor("wq", [P, KC, GE], BF, kind="ExternalInput")
    d_wk = nc.dram_tensor("wk", [P, KC, GE], BF, kind="ExternalInput")
    d_wv = nc.dram_tensor("wv", [P, KC, GE], BF, kind="ExternalInput")
    d_wo = nc.dram_tensor("wo", [P, 2, E], BF, kind="ExternalInput")
    d_bq = nc.dram_tensor("bqs", [P, 2], F32, kind="ExternalInput")
    d_bk = nc.dram_tensor("bks", [P, 2], F32, kind="ExternalInput")
    d_bv = nc.dram_tensor("bvb", [P, GE], F32, kind="ExternalInput")
    d_bo = nc.dram_tensor("bob", [P, E], F32, kind="ExternalInput")
    d_y = nc.dram_tensor("y", [S, E], F32, kind="ExternalOutput")

    with tile.TileContext(nc) as tc:
        with (
            tc.tile_pool(name="w", bufs=1) as wpool,
            tc.tile_pool(name="x", bufs=1) as xpool,
            tc.tile_pool(name="persist", bufs=1) as pers,
            tc.tile_pool(name="pt", bufs=16) as ptp,
            tc.tile_pool(name="sm", bufs=2) as sm,
            tc.tile_pool(name="y", bufs=2) as yp,
            tc.tile_pool(name="psA", bufs=2, space="PSUM") as psA,
            tc.tile_pool(name="psB", bufs=4, space="PSUM") as psB,
        ):
            # ---- weights / biases resident in SBUF ----
            wq_t = wpool.tile([P, KC, GE], BF, tag="wq")
            wk_t = wpool.tile([P, KC, GE], BF, tag="wk")
            wv_t = wpool.tile([P, KC, GE], BF, tag="wv")
            wo_t = wpool.tile([P, 2, E], BF, tag="wo")
            bq_t = wpool.tile([P, 2], F32, tag="bq")
            bk_t = wpool.tile([P, 2], F32, tag="bk")
            bv_t = wpool.tile([P, GE], F32, tag="bv")
            bo_t = wpool.tile([P, E], F32, tag="bo")
            ones_t = wpool.tile([P, 64], F32, tag="ones")
            nc.vector.memset(ones_t[:], 1.0)

            # prologue-critical weights first; wv/wo/biases are loaded later
            # (between the prologue slice-prefetches and the xv chunks) so
            # the first projection MMs are not queued behind cold data
            for t, d in ((wk_t, d_wk), (wq_t, d_wq)):
                nc.sync.dma_start(t[:], d[:])

            # ---- persistent activations ----
            QT = pers.tile([P, 2, S], BF, tag="QT")   # [d'(2x128), S]
            KT = pers.tile([P, 2, S], BF, tag="KT")
            V1 = pers.tile([P, JC, 260], BF, tag="V1")  # [S(16x128), (V_h|1)*4]
            OT = pers.tile([P, 2, S], BF, tag="OT")

            # ones columns (col 64 of each 65-wide head block)
            nc.vector.memset(V1[:, :, 64::65], 1.0)



            def qk_steps(w_t, b_t, dst, xd, c, tt, th):
                # one [128, 512] span of a Q/K projection as a prefetch
                # (1 MB strided slice DMA) plus 4 steps of 2 accumulating
                # MMs; the last step adds the bias on copy-back. Psum from
                # the 1-bank rotating pool so the logits double-buffer is
                # never starved.
                s0 = (tt * 2 + th) * 512
                st = {}

                def prefetch():
                    st["xs"] = xpool.tile([P, KC, 512], BF, tag="xs", bufs=4,
                                          name=f"xs_{c}_{tt}_{th}")
                    nc.sync.dma_start(st["xs"][:], xd[tt * 2 + th])

                def step(i):
                    if i == 0:
                        st["ps"] = psB.tile([P, 512], F32, tag="acc",
                                            name=f"qkps_{c}_{tt}_{th}")
                    for kc in (2 * i, 2 * i + 1):
                        nc.tensor.matmul(
                            st["ps"][:],
                            lhsT=w_t[:, kc, c * P:(c + 1) * P],
                            rhs=st["xs"][:, kc, :],
                            start=(kc == 0), stop=(kc == KC - 1),
                        )
                    if i == 3:
                        nc.vector.tensor_scalar_add(
                            dst[:, c, s0:s0 + 512], st["ps"][:],
                            b_t[:, c:c + 1])

                steps = []
                for i in range(4):
                    steps += [lambda i=i: step(i), None]
                return [prefetch], steps

            def v_steps(sg):
                # V projection for two s-chunks -> V1 (interleaved V|1 cols):
                # a 0.5 MB just-in-time slice prefetch plus 2 steps of 8 MMs,
                # each step copying back its own s-chunk (so PV(jc) may run as
                # soon as the slot-jc step has finished).
                st = {}

                def prefetch():
                    st["vs"] = xpool.tile([P, KC, GE], BF, tag="vs", bufs=4,
                                          name=f"vs_{sg}")
                    nc.sync.dma_start(st["vs"][:], d_xv[sg])

                def step(i2):
                    if i2 == 0:
                        st["ps"] = psB.tile([P, 512], F32, tag="acc",
                                            name=f"vps_{sg}")
                    sc = sg * 2 + i2
                    for kc in range(KC):
                        nc.tensor.matmul(
                            st["ps"][:, i2 * GE:(i2 + 1) * GE],
                            lhsT=st["vs"][:, kc, i2 * P:(i2 + 1) * P],
                            rhs=wv_t[:, kc, :],
                            start=(kc == 0), stop=(kc == KC - 1),
                        )
                    for h in range(4):
                        nc.vector.tensor_tensor(
                            V1[:, sc, 65 * h:65 * h + 64],
                            st["ps"][:, i2 * GE + 64 * h:
                                     i2 * GE + 64 * (h + 1)],
                            bv_t[:, 64 * h:64 * (h + 1)],
                            add,
                        )

                return [prefetch], [lambda i=i: step(i) for i in range(2)]

            def out_group(sc, nt, ysb):
                ps = psB.tile([P, 512], F32, tag="acc")
                for cc in range(2):
                    nc.tensor.matmul(
                        ps[:],
                        lhsT=OT[:, cc, sc * P:(sc + 1) * P],
                        rhs=wo_t[:, cc, nt * 512:(nt + 1) * 512],
                        start=(cc == 0), stop=(cc == 1),
                    )
                nc.vector.tensor_tensor(
                    ysb[:, nt * 512:(nt + 1) * 512], ps[:],
                    bo_t[:, nt * 512:(nt + 1) * 512], add)

            # ---- prologue: all of K(c=0) and Q(c=0, tt0, th0); everything
            # else (V included) rides as fillers under the exp stream ----
            pro = [qk_steps(wk_t, bk_t, KT, d_xk, 0, tt, th)
                   for tt in range(2) for th in range(2)]
            pro += [qk_steps(wq_t, bq_t, QT, d_xq, 0, 0, 0)]
            for pre, _ in pro:       # issue all slice DMAs up front
                for p in pre:
                    p()
            # then queue the remaining small input loads
            for t, d in ((bk_t, d_bk), (bq_t, d_bq), (wv_t, d_wv),
                         (bv_t, d_bv), (wo_t, d_wo), (bo_t, d_bo)):
                nc.sync.dma_start(t[:], d[:])
            for _, steps in pro:
                for s in steps:
                    if s is not None:
                        s()

            # ---- attention, ACT(exp)-bound; remaining projection and the
            # output-projection work is sprinkled between jc iterations so it
            # fills the PE's idle capacity without starving the exp stream ----
            def attn_unit(c, t, groups):
                # groups: list of (prefetches, steps); prefetches (DMAs) are
                # issued at unit start, one step (~2-8 MMs) runs per jc.
                fillers = []
                for pre, steps in groups:
                    for p in pre:
                        p()
                    fillers.extend(steps)
                tsl = slice(t * 512, (t + 1) * 512)
                pO0 = psB.tile([65, 512], F32, tag="acc")
                pO1 = psB.tile([65, 512], F32, tag="acc")
                for jc in range(JC):
                    if fillers:
                        f = fillers.pop(0)
                        if f is not None:
                            f()
                    jsl = slice(jc * P, (jc + 1) * P)
                    pL = psA.tile([P, 1024], F32, tag="big")
                    nc.tensor.matmul(
                        pL[:, 0:512],
                        lhsT=KT[0:64, c, jsl], rhs=QT[0:64, c, tsl],
                        start=True, stop=True,
                    )
                    nc.tensor.matmul(
                        pL[:, 512:1024],
                        lhsT=KT[64:128, c, jsl], rhs=QT[64:128, c, tsl],
                        start=True, stop=True,
                    )
                    pt = ptp.tile([P, 1024], BF, tag="pt")
                    nc.scalar.activation(pt[:], pL[:], Exp)
                    nc.tensor.matmul(
                        pO0[:], lhsT=V1[:, jc, 130 * c:130 * c + 65],
                        rhs=pt[:, 0:512],
                        start=(jc == 0), stop=(jc == JC - 1),
                    )
                    nc.tensor.matmul(
                        pO1[:], lhsT=V1[:, jc, 130 * c + 65:130 * c + 130],
                        rhs=pt[:, 512:1024],
                        start=(jc == 0), stop=(jc == JC - 1),
                    )
                # normalize: OT_h = pO[0:64] / pO[64]  (row 64 = sum of P).
                # Copy the accumulator to SBUF (frees the bank, and gives the
                # multiply an SBUF operand), take the exact reciprocal of the
                # sum row in place on the DVE, and broadcast it across 64
                # partitions with a K=1 PE outer-product (ones x 1/S).
                for hp, pO in ((0, pO0), (1, pO1)):
                    osb = sm.tile([65, 512], F32, tag="osb", bufs=3)
                    nc.vector.tensor_copy(osb[:], pO[:])
                    rec0 = sm.tile([1, 512], F32, tag="rec0", bufs=3)
                    nc.sync.dma_start(rec0[:], osb[64:65, :])
                    rin = sm.tile([1, 512], F32, tag="rin", bufs=3)
                    nc.vector.reciprocal_approx_fast(rin[:], rec0[:])
                    rbs = sm.tile([64, 512], F32, tag="rbs", bufs=3)
                    nc.gpsimd.partition_broadcast(rbs[:], rin[:])
                    ott = sm.tile([64, 512], BF, tag="ott", bufs=3)
                    nc.vector.tensor_tensor(ott[:], osb[0:64, :],
                                            rbs[:], mult)
                    nc.sync.dma_start(OT[64 * hp:64 * (hp + 1), c, tsl],
                                      ott[:])
                while fillers:
                    f = fillers.pop(0)
                    if f is not None:
                        f()

            ysbs = {}

            def mkout(sc, nt):
                def f():
                    if sc not in ysbs:
                        ysbs[sc] = yp.tile([P, E], F32, tag="ysb",
                                           name=f"ysb_{sc}")
                    out_group(sc, nt, ysbs[sc])
                    if nt == 1:
                        nc.sync.dma_start(
                            d_y[sc * P:(sc + 1) * P, :], ysbs.pop(sc)[:])
                return f

            # c=0 units carry the remaining projections; c=1 units carry the
            # output projection of a PREVIOUSLY finished i-tile (Tile orders
            # dataflow by emission, so a filler may only read regions whose
            # writes were already emitted).
            def outfill(t):
                # two empty lead slots let the exp stream buffer up before
                # the first 2-MM out-projection burst lands in the PE queue
                return [([], [None, None])] + [
                    ([], [mkout(sc, nt), None])
                    for sc in range(4 * t, 4 * t + 4) for nt in range(2)]

            attn_unit(0, 0, [v_steps(sg) for sg in range(8)]
                      + [qk_steps(wq_t, bq_t, QT, d_xq, 0, 0, 1)])
            attn_unit(0, 1, [qk_steps(wq_t, bq_t, QT, d_xq, 0, 1, 0),
                             qk_steps(wq_t, bq_t, QT, d_xq, 0, 1, 1)])
            attn_unit(0, 2, [qk_steps(wk_t, bk_t, KT, d_xk, 1, 0, 0),
                             qk_steps(wk_t, bk_t, KT, d_xk, 1, 0, 1),
                             qk_steps(wq_t, bq_t, QT, d_xq, 1, 0, 0)])
            attn_unit(0, 3, [qk_steps(wk_t, bk_t, KT, d_xk, 1, 1, 0),
                             qk_steps(wk_t, bk_t, KT, d_xk, 1, 1, 1),
                             qk_steps(wq_t, bq_t, QT, d_xq, 1, 0, 1)])
            attn_unit(1, 0, [qk_steps(wq_t, bq_t, QT, d_xq, 1, 1, 0),
                             qk_steps(wq_t, bq_t, QT, d_xq, 1, 1, 1)])
            attn_unit(1, 1, outfill(0))
            attn_unit(1, 2, outfill(1))
            attn_unit(1, 3, outfill(2))
            for _, steps in outfill(3):
                for s in steps:
                    if s is not None:
                        s()

    nc.compile()
    return nc


def _get_program():
    global _NC
    if _NC is None:
        _NC = _build_program()
    return _NC


def kernel(q, k, v, Wq, bq, Wk, bk, Wv, bv, Wo, bo):
    from concourse.bass_utils import run_bass_kernel_spmd

    q = np.asarray(q, np.float32)
    k = np.asarray(k, np.float32)
    v = np.asarray(v, np.float32)
    Wq = np.asarray(Wq, np.float32)
    Wk = np.asarray(Wk, np.float32)
    Wv = np.asarray(Wv, np.float32)
    Wo = np.asarray(Wo, np.float32)
    bq = np.asarray(bq, np.float32)
    bk = np.asarray(bk, np.float32)
    bv = np.asarray(bv, np.float32)
    bo = np.asarray(bo, np.float32)

    nc = _get_program()

    def tile_qk(xb):
        # [S, E] -> x^T tiled [4, 128, KC, 512]
        return np.ascontiguousarray(
            xb.T.reshape(KC, P, 4, 512).transpose(2, 1, 0, 3)).astype(BF16)

    def tile_v(xb):
        # [S, E] -> x^T tiled [8, 128, KC, 256]
        return np.ascontiguousarray(
            xb.T.reshape(KC, P, 8, GE).transpose(2, 1, 0, 3)).astype(BF16)

    xT = {"xqT": [tile_qk(q[b]) for b in range(2)],
          "xkT": [tile_qk(k[b]) for b in range(2)],
          "xvT": [tile_v(v[b]) for b in range(2)]}

    def wprep(W, scale=1.0):
        # [E, GE] slice -> [P, KC, GE] partition-major
        return [
            np.ascontiguousarray(
                (W[:, g * GE:(g + 1) * GE] * scale)
                .reshape(KC, P, GE).transpose(1, 0, 2)
            ).astype(BF16)
            for g in range(4)
        ]

    wq_g = wprep(Wq, 0.125)
    wk_g = wprep(Wk)
    wv_g = wprep(Wv)
    wo_g = [
        np.ascontiguousarray(
            Wo[g * GE:(g + 1) * GE, :].reshape(2, P, E).transpose(1, 0, 2)
        ).astype(BF16)
        for g in range(4)
    ]
    bq_g = [np.ascontiguousarray((bq[g * GE:(g + 1) * GE] * 0.125)
                                 .reshape(2, P).T).astype(np.float32)
            for g in range(4)]
    bk_g = [np.ascontiguousarray(bk[g * GE:(g + 1) * GE].reshape(2, P).T)
            .astype(np.float32) for g in range(4)]
    bv_g = [np.ascontiguousarray(np.broadcast_to(
        bv[g * GE:(g + 1) * GE].astype(np.float32), (P, GE))) for g in range(4)]
    bo_full = np.ascontiguousarray(
        np.broadcast_to(bo.astype(np.float32), (P, E)))
    bo_zero = np.zeros((P, E), np.float32)

    in_maps = []
    for c in range(NCORES):
        b, g = divmod(c, 4)
        in_maps.append({
            "xqT": xT["xqT"][b],
            "xkT": xT["xkT"][b],
            "xvT": xT["xvT"][b],
            "wq": wq_g[g], "wk": wk_g[g], "wv": wv_g[g], "wo": wo_g[g],
            "bqs": bq_g[g], "bks": bk_g[g], "bvb": bv_g[g],
            "bob": bo_full if g == 0 else bo_zero,
        })

    res = run_bass_kernel_spmd(nc, in_maps, list(range(NCORES)),
                               **_RUN_KWARGS)
    globals()["LAST_RESULTS"] = res

    parts = [res.results[c]["y"] for c in range(NCORES)]
    out = np.stack([
        parts[0] + parts[1] + parts[2] + parts[3],
        parts[4] + parts[5] + parts[6] + parts[7],
    ]).astype(np.float32)
    return out


# test-harness hooks (kernel.py itself never enables tracing)
_RUN_KWARGS = {}
LAST_RESULTS = None
